# revision 2
# baseline (speedup 1.0000x reference)
"""Trainium2 Bass kernel for MultiHeadEdgeAwareMessagePassing.

Math restructure (validated vs reference on host, final rel err ~1e-3 incl.
device exp table):
  logits[i,j,h] = s_q[i,h] + s_k[j,h] + w[i,j]*c1[h] + c0[h]   (valid j: w>0)
  alpha = softmax_j(logits) * w
s_q, c0 are constant over j and cancel in the softmax; bq/bk contributions
cancel exactly.  With g[j,h] = exp(h[j]@a_k[:,h]), a_k = per-head u_k @ Wk
(host-folded weight constant), v = h@Wv^T:
  msg[i,h,:] = (Num_h[i,:] + bv_h * Ag_h[i]) / Den_h[i]
  Num_h = W1^T (g_h*v_h)      Ag_h = W1^T g_h
  Den_h = mask^T g_h + c1_h * Ag_h
where mask=[w>0], W1=relu(w)  (exp(c1 w) ~= 1 + c1 w; dropped quadratic
term changes the final output by ~3e-6 relative).

Sharding: destination rows i split across 8 cores (384 rows each). Each core
reads its [3072, 384] slice of w^T (bf16) plus replicated h^T and the small
weights. Host-side work is layout/dtype prep and weight-only constant
folding; all data compute runs on device.
"""

import numpy as np

N = 3072
D = 256
H = 4
DH = 64
DE = 8
NCORES = 8
ISLICE = N // NCORES  # 384
NSUB = ISLICE // 128  # 3
CJT = 4               # j-tiles per chunk
NCH = N // (128 * CJT)  # 6 chunks

# su2 packed bf16 column offsets
S2_WOT = 0          # 512: WoT as [p, a, 256]
S2_ID = 512         # 128: identity
S2_BO = 640         # 256: bo row (partition 0)
S2_C1 = 896         # 4:   c1 broadcast [128, 4]
S2_BV = 900         # 256: bv broadcast [128, 256]
S2_GAM = 1156       # 256: gamma broadcast
S2_BET = 1412       # 256: beta broadcast
S2_COLS = 1668

_cache = {}


def _build_bass():
    import concourse.bass as bass
    import concourse.tile as tile
    from concourse import bacc, mybir
    from concourse.bass import ts

    dt = mybir.dt
    AF = mybir.ActivationFunctionType
    OP = mybir.AluOpType

    nc = bacc.Bacc("TRN2", target_bir_lowering=False, debug=False,
                   num_devices=NCORES)

    wt_d = nc.dram_tensor("wt", [N, ISLICE], dt.bfloat16, kind="ExternalInput")
    ht_d = nc.dram_tensor("ht", [D, N], dt.bfloat16, kind="ExternalInput")
    hs_d = nc.dram_tensor("hs", [ISLICE, D], dt.float32, kind="ExternalInput")
    # su1: [128, 2, 260] bf16: per d-half a: WvT block (256) | a_k block (4)
    su1_d = nc.dram_tensor("su1", [128, 2 * 260], dt.bfloat16,
                           kind="ExternalInput")
    su2_d = nc.dram_tensor("su2", [128, S2_COLS], dt.bfloat16,
                           kind="ExternalInput")
    out_d = nc.dram_tensor("out", [ISLICE, D], dt.float32,
                           kind="ExternalOutput")

    bf = dt.bfloat16
    f8 = dt.float8e4
    f32 = dt.float32

    with tile.TileContext(nc) as tc:
        with (
            tc.tile_pool(name="consts", bufs=1) as consts,
            tc.tile_pool(name="wtp", bufs=3) as wtp,
            tc.tile_pool(name="elem", bufs=3) as elem,
            tc.tile_pool(name="rhsp", bufs=3) as rhsp,
            tc.tile_pool(name="small", bufs=12) as small,
            tc.tile_pool(name="outp", bufs=9) as outp,
            tc.tile_pool(name="acc", bufs=1, space="PSUM") as accp,
            tc.tile_pool(name="pre", bufs=4, space="PSUM") as prep,
        ):
            # ---------------- consts ----------------
            su1 = consts.tile([128, 2, 260], bf, tag="su1")
            nc.sync.dma_start(su1, su1_d.ap().rearrange(
                "p (a n) -> p a n", a=2))
            ones_sb = consts.tile([1, 128], bf, tag="ones")
            nc.vector.memset(ones_sb, 1.0)
            eps_sb = consts.tile([128, 1], f32, tag="eps")
            nc.vector.memset(eps_sb, 1e-5)

            # persistent accumulators: 0:256 Num | 256:260 Ag | 260:264 Amask
            psA = [accp.tile([128, 264], f32, tag=f"A{s}", name=f"psA{s}")
                   for s in range(NSUB)]

            # ---------------- bulk DMAs ----------------
            ht_sb = consts.tile([128, 2, N], bf, tag="ht")
            ht_re = ht_d.ap().rearrange("(a p) n -> p a n", p=128)
            wt_tiles = []
            for ch in range(NCH):
                wt_tiles.append(wtp.tile([128, CJT, ISLICE], bf, tag="wt",
                                         name=f"wt4_{ch}"))
            for ch in range(NCH):
                nc.sync.dma_start(ht_sb[:, :, ts(ch, 128 * CJT)],
                                  ht_re[:, :, ts(ch, 128 * CJT)])
                nc.sync.dma_start(
                    wt_tiles[ch], wt_d[ts(ch, 128 * CJT), :].rearrange(
                        "(j p) i -> p j i", p=128))
            hseg_all = consts.tile([128, NSUB, D], f32, tag="hsegall")
            nc.sync.dma_start(
                hseg_all, hs_d.ap().rearrange("(s p) n -> p s n", p=128))
            su2 = consts.tile([128, S2_COLS], bf, tag="su2")
            nc.sync.dma_start(su2, su2_d.ap())

            WoT_sb = su2[:, S2_WOT:S2_WOT + 512].rearrange(
                "p (a n) -> p a n", a=2)
            ident = su2[:, S2_ID:S2_ID + 128]
            bo_row = su2[0:1, S2_BO:S2_BO + 256]
            c1b_sb = su2[:, S2_C1:S2_C1 + 4]
            bv_sb = su2[:, S2_BV:S2_BV + 256]
            gam_sb = su2[:, S2_GAM:S2_GAM + 256]
            bet_sb = su2[:, S2_BET:S2_BET + 256]

            # ---------------- main loop (software pipelined) ----------------
            def emit_front(ch):
                """relu/mask + projections (v, s_k, g, g*v) for chunk ch."""
                wt4 = wt_tiles[ch]
                W1c = elem.tile([128, CJT, ISLICE], bf, tag="W1",
                                name=f"W1_{ch}")
                nc.gpsimd.tensor_scalar_max(W1c, wt4, 0.0)
                mskc = elem.tile([128, CJT, ISLICE], f8, tag="msk",
                                 name=f"msk_{ch}")
                nc.vector.tensor_scalar(mskc, wt4, 0.0, None, op0=OP.is_gt)

                rhs4 = rhsp.tile([128, CJT, 260], bf, tag="rhs",
                                 name=f"rhs_{ch}")
                for jm in range(CJT):
                    jt = ch * CJT + jm
                    ps_vk = prep.tile([128, 260], f32, tag="vk",
                                      name=f"vk_{jt % 4}")
                    for a in range(2):
                        nc.tensor.matmul(ps_vk, ht_sb[:, a, ts(jt, 128)],
                                         su1[:, a, :],
                                         start=(a == 0), stop=(a == 1))
                    # g = exp(s_k) written straight into rhs cols 256:260
                    nc.scalar.activation(rhs4[:, jm, 256:260],
                                         ps_vk[:, 256:260], AF.Exp)
                    gsl = rhs4[:, jm, 256:260]
                    gb = bass.AP(tensor=gsl.tensor, offset=gsl.offset,
                                 ap=[gsl.ap[0], gsl.ap[1], [0, DH]])
                    nc.vector.tensor_tensor(
                        out=rhs4[:, jm, 0:256].rearrange(
                            "p (h d) -> p h d", h=H),
                        in0=ps_vk[:, 0:256].rearrange("p (h d) -> p h d", h=H),
                        in1=gb, op=OP.mult)
                return W1c, mskc, rhs4

            def emit_accum(ch, W1c, mskc, rhs4):
                st = (ch == 0)
                sp = (ch == NCH - 1)
                for jm in range(CJT):
                    for s in range(NSUB):
                        nc.tensor.matmul(psA[s][:, 0:260],
                                         W1c[:, jm, ts(s, 128)],
                                         rhs4[:, jm, :], start=st, stop=sp,
                                         skip_group_check=True)
                for jm in range(CJT):
                    for s in range(NSUB):
                        nc.tensor.matmul(psA[s][:, 260:264],
                                         mskc[:, jm, ts(s, 128)],
                                         rhs4[:, jm, 256:260], start=st,
                                         stop=sp, skip_group_check=True)

            prev = emit_front(0)
            for ch in range(1, NCH):
                cur = emit_front(ch)
                emit_accum(ch - 1, *prev)
                prev = cur
            emit_accum(NCH - 1, *prev)

            # ---------------- epilogue ----------------
            rdens, agrs = [], []
            for s in range(NSUB):
                den = small.tile([128, H], f32, tag="den", name=f"den{s}")
                nc.vector.tensor_tensor(den, psA[s][:, 256:260], c1b_sb,
                                        op=OP.mult)
                nc.vector.tensor_add(den, den, psA[s][:, 260:264])
                rden = small.tile([128, H], f32, tag="rden", name=f"rden{s}")
                nc.vector.reciprocal(rden, den)
                agr = small.tile([128, H], f32, tag="agr", name=f"agr{s}")
                nc.vector.tensor_mul(agr, psA[s][:, 256:260], rden)
                rdens.append(rden)
                agrs.append(agr)

            msgs = []
            for s in range(NSUB):
                msg = outp.tile([128, D], bf, tag="msg", name=f"msg{s}")
                rd = rdens[s]
                rdb = bass.AP(tensor=rd.tensor, offset=rd.offset,
                              ap=[rd.ap[0], rd.ap[1], [0, DH]])
                nc.vector.tensor_tensor(
                    out=msg.rearrange("p (h d) -> p h d", h=H),
                    in0=psA[s][:, 0:256].rearrange("p (h d) -> p h d", h=H),
                    in1=rdb, op=OP.mult)
                # bv fold: msg += bv * (Ag/den)  (gpsimd, SBUF-only)
                ag = agrs[s]
                agb = bass.AP(tensor=ag.tensor, offset=ag.offset,
                              ap=[ag.ap[0], ag.ap[1], [0, DH]])
                msgb = outp.tile([128, D], bf, tag="msgb", name=f"msgb{s}")
                nc.gpsimd.tensor_tensor(
                    out=msgb.rearrange("p (h d) -> p h d", h=H),
                    in0=bv_sb.rearrange("p (h d) -> p h d", h=H),
                    in1=agb, op=OP.mult)
                msg2 = outp.tile([128, D], bf, tag="msg2", name=f"msg2{s}")
                nc.gpsimd.tensor_add(msg2, msg, msgb)
                msgs.append(msg2)

            msgTs = []
            for s in range(NSUB):
                ps_t = prep.tile([128, 2, 128], bf, tag="vk", name=f"pst{s}")
                for b in range(2):
                    nc.tensor.transpose(ps_t[:, b, :], msgs[s][:, ts(b, 128)],
                                        ident)
                msgT = outp.tile([128, 2, 128], bf, tag="msgT",
                                 name=f"msgT{s}")
                nc.vector.tensor_copy(msgT, ps_t)
                msgTs.append(msgT)

            for s in range(NSUB):
                ps_o = prep.tile([128, D], f32, tag="vk", name=f"pso{s}")
                nc.tensor.matmul(ps_o, msgTs[s][:, 0, :], WoT_sb[:, 0, :],
                                 start=True, stop=False)
                nc.tensor.matmul(ps_o, msgTs[s][:, 1, :], WoT_sb[:, 1, :],
                                 start=False, stop=False)
                nc.tensor.matmul(ps_o, ones_sb, bo_row, start=False,
                                 stop=True)

                x = outp.tile([128, D], f32, tag="x", name=f"x{s}")
                nc.vector.tensor_add(x, ps_o, hseg_all[:, s, :])

                stats = small.tile([128, 6], f32, tag="stats", name=f"st{s}")
                nc.vector.bn_stats(out=stats, in_=x)
                mv = small.tile([128, 2], f32, tag="mv", name=f"mv{s}")
                nc.vector.bn_aggr(out=mv, in_=stats)
                sd = small.tile([128, 1], f32, tag="sd", name=f"sd{s}")
                nc.scalar.activation(sd, mv[:, 1:2], AF.Sqrt, bias=eps_sb)
                rstd = small.tile([128, 1], f32, tag="rstd", name=f"rst{s}")
                nc.vector.reciprocal(rstd, sd)

                # G = gamma * rstd  (per-partition scalar mult, gpsimd)
                G = outp.tile([128, D], f32, tag="G", name=f"G{s}")
                nc.gpsimd.tensor_scalar(G, gam_sb, rstd, None, op0=OP.mult)
                # ot = (x - mu) * G
                ot = outp.tile([128, D], f32, tag="ot", name=f"ot{s}")
                nc.vector.scalar_tensor_tensor(
                    out=ot, in0=x, scalar=mv[:, 0:1], in1=G,
                    op0=OP.subtract, op1=OP.mult)
                otb = outp.tile([128, D], f32, tag="otb", name=f"otb{s}")
                nc.gpsimd.tensor_add(otb, ot, bet_sb)
                nc.sync.dma_start(out_d[ts(s, 128), :], otb)

    nc.compile()
    return nc


def _make_in_maps(h, w, Wk, Wv, bv, We_w, u, Wo, bo, gamma, beta, **_unused):
    import ml_dtypes
    f = np.float32
    b16 = ml_dtypes.bfloat16
    h32 = np.ascontiguousarray(h, dtype=f)
    wT = np.asarray(w, dtype=f).T.astype(b16)
    Wk = np.asarray(Wk, dtype=f)
    u = np.asarray(u, dtype=f)
    We_w = np.asarray(We_w, dtype=f)

    # host-folded weight constants
    u_k, u_e = u[:, DH:2 * DH], u[:, 2 * DH:2 * DH + DE]
    a_k = np.stack([u_k[hh] @ Wk[hh * DH:(hh + 1) * DH, :]
                    for hh in range(H)], axis=1)          # [256, 4]
    c1 = np.array([We_w[hh * DE:(hh + 1) * DE, 0] @ u_e[hh]
                   for hh in range(H)], dtype=f)          # [4]

    # su1 [128, 2, 260]: per d-half a: WvT block | a_k block
    su1 = np.zeros((128, 2, 260), f)
    WvT = np.asarray(Wv, dtype=f).T                        # [din, dout]
    su1[:, :, 0:256] = WvT.reshape(2, 128, D).transpose(1, 0, 2)
    su1[:, :, 256:260] = a_k.reshape(2, 128, H).transpose(1, 0, 2)

    # su2 [128, S2_COLS]
    su2 = np.zeros((128, S2_COLS), f)
    WoT = np.asarray(Wo, dtype=f).T
    su2[:, S2_WOT:S2_WOT + 512] = WoT.reshape(2, 128, D).transpose(
        1, 0, 2).reshape(128, 512)
    su2[:, S2_ID:S2_ID + 128] = np.eye(128, dtype=f)
    su2[0, S2_BO:S2_BO + 256] = np.asarray(bo, dtype=f)
    su2[:, S2_C1:S2_C1 + 4] = c1[None, :]
    su2[:, S2_BV:S2_BV + 256] = np.asarray(bv, dtype=f)[None, :]
    su2[:, S2_GAM:S2_GAM + 256] = np.asarray(gamma, dtype=f)[None, :]
    su2[:, S2_BET:S2_BET + 256] = np.asarray(beta, dtype=f)[None, :]

    common = {
        "ht": np.ascontiguousarray(h32.T.astype(b16)),
        "su1": su1.reshape(128, 520).astype(b16),
        "su2": su2.astype(b16),
    }
    in_maps = []
    for c in range(NCORES):
        sl = slice(c * ISLICE, (c + 1) * ISLICE)
        m = dict(common)
        m["wt"] = np.ascontiguousarray(wT[:, sl])
        m["hs"] = np.ascontiguousarray(h32[sl, :])
        in_maps.append(m)
    return in_maps


def kernel(**inputs):
    from concourse.bass_utils import run_bass_kernel_spmd

    if "nc" not in _cache:
        _cache["nc"] = _build_bass()
    nc = _cache["nc"]

    in_maps = _make_in_maps(**inputs)
    res = run_bass_kernel_spmd(nc, in_maps, core_ids=list(range(NCORES)))
    out = np.concatenate([r["out"] for r in res.results], axis=0)
    return np.ascontiguousarray(out, dtype=np.float32)


# revision 6
# speedup vs baseline: 3.5047x; 3.5047x over previous
"""Trainium2 Bass kernel for MultiHeadEdgeAwareMessagePassing.

Math restructure (validated vs reference on host, final rel err ~1e-3 incl.
device exp table):
  logits[i,j,h] = s_q[i,h] + s_k[j,h] + w[i,j]*c1[h] + c0[h]   (valid j: w>0)
  alpha = softmax_j(logits) * w
s_q, c0 are constant over j and cancel in the softmax; bq/bk contributions
cancel exactly.  With g[j,h] = exp(h[j]@a_k[:,h]), a_k = per-head u_k @ Wk
(host-folded weight constant), v = h@Wv^T:
  msg[i,h,:] = (Num_h[i,:] + bv_h * Ag_h[i]) / Den_h[i]
  Num_h = W1^T (g_h*v_h)      Ag_h = W1^T g_h
  Den_h = mask^T g_h + c1_h * Ag_h
where mask=[w>0], W1=relu(w)  (exp(c1 w) ~= 1 + c1 w; dropped quadratic
term changes the final output by ~3e-6 relative).

Sharding: destination rows i split across 8 cores (384 rows each). Each core
reads its [3072, 384] slice of w^T (bf16) plus replicated h^T and the small
weights. Host-side work is layout/dtype prep and weight-only constant
folding; all data compute runs on device.
"""

import numpy as np

N = 3072
D = 256
H = 4
DH = 64
DE = 8
NCORES = 8
ISLICE = N // NCORES  # 384
NSUB = ISLICE // 128  # 3
CJT = 4               # j-tiles per chunk
NCH = N // (128 * CJT)  # 6 chunks

# su2 packed bf16 column offsets
S2_WOT = 0          # 512: WoT as [p, a, 256]
S2_ID = 512         # 128: identity
S2_BO = 640         # 256: bo row (partition 0)
S2_C1 = 896         # 4:   c1 broadcast [128, 4]
S2_BV = 900         # 256: bv broadcast [128, 256]
S2_GAM = 1156       # 256: gamma broadcast
S2_BET = 1412       # 256: beta broadcast
S2_COLS = 1668

_cache = {}


def _build_bass():
    import concourse.bass as bass
    import concourse.tile as tile
    from concourse import bacc, mybir
    from concourse.bass import ts

    dt = mybir.dt
    AF = mybir.ActivationFunctionType
    OP = mybir.AluOpType

    nc = bacc.Bacc("TRN2", target_bir_lowering=False, debug=False,
                   num_devices=NCORES)

    wt_d = nc.dram_tensor("wt", [N, ISLICE], dt.bfloat16, kind="ExternalInput")
    ht_d = nc.dram_tensor("ht", [D, N], dt.bfloat16, kind="ExternalInput")
    hs_d = nc.dram_tensor("hs", [ISLICE, D], dt.float32, kind="ExternalInput")
    # su1: [128, 2, 260] bf16: per d-half a: WvT block (256) | a_k block (4)
    su1_d = nc.dram_tensor("su1", [128, 2 * 260], dt.bfloat16,
                           kind="ExternalInput")
    su2_d = nc.dram_tensor("su2", [128, S2_COLS], dt.bfloat16,
                           kind="ExternalInput")
    out_d = nc.dram_tensor("out", [ISLICE, D], dt.float32,
                           kind="ExternalOutput")

    bf = dt.bfloat16
    f32 = dt.float32

    with tile.TileContext(nc) as tc:
        with (
            tc.tile_pool(name="consts", bufs=1) as consts,
            tc.tile_pool(name="wtp", bufs=3) as wtp,
            tc.tile_pool(name="elem", bufs=3) as elem,
            tc.tile_pool(name="rhsp", bufs=3) as rhsp,
            tc.tile_pool(name="small", bufs=12) as small,
            tc.tile_pool(name="outp", bufs=9) as outp,
            tc.tile_pool(name="acc", bufs=1, space="PSUM") as accp,
            tc.tile_pool(name="pre", bufs=4, space="PSUM") as prep,
        ):
            # ---------------- consts ----------------
            su1 = consts.tile([128, 2, 260], bf, tag="su1")
            nc.sync.dma_start(su1, su1_d.ap().rearrange(
                "p (a n) -> p a n", a=2))
            ones_sb = consts.tile([1, 128], bf, tag="ones")
            nc.vector.memset(ones_sb, 1.0)
            eps_sb = consts.tile([128, 1], f32, tag="eps")
            nc.vector.memset(eps_sb, 1e-5)

            # persistent accumulators: 0:256 Num | 256:260 Ag | 260:264 Amask
            psA = [accp.tile([128, 264], f32, tag=f"A{s}", name=f"psA{s}")
                   for s in range(NSUB)]

            # ---------------- bulk DMAs ----------------
            ht_sb = consts.tile([128, 2, N], bf, tag="ht")
            ht_re = ht_d.ap().rearrange("(a p) n -> p a n", p=128)
            wt_tiles = []
            for ch in range(NCH):
                wt_tiles.append(wtp.tile([128, CJT, ISLICE], bf, tag="wt",
                                         name=f"wt4_{ch}"))
            for ch in range(NCH):
                nc.sync.dma_start(ht_sb[:, :, ts(ch, 128 * CJT)],
                                  ht_re[:, :, ts(ch, 128 * CJT)])
                nc.sync.dma_start(
                    wt_tiles[ch], wt_d[ts(ch, 128 * CJT), :].rearrange(
                        "(j p) i -> p j i", p=128))
            hseg_all = consts.tile([128, NSUB, D], f32, tag="hsegall")
            nc.sync.dma_start(
                hseg_all, hs_d.ap().rearrange("(s p) n -> p s n", p=128))
            su2 = consts.tile([128, S2_COLS], bf, tag="su2")
            nc.sync.dma_start(su2, su2_d.ap())

            WoT_sb = su2[:, S2_WOT:S2_WOT + 512].rearrange(
                "p (a n) -> p a n", a=2)
            ident = su2[:, S2_ID:S2_ID + 128]
            bo_row = su2[0:1, S2_BO:S2_BO + 256]
            c1b_sb = su2[:, S2_C1:S2_C1 + 4]
            bv_sb = su2[:, S2_BV:S2_BV + 256]
            gam_sb = su2[:, S2_GAM:S2_GAM + 256]
            bet_sb = su2[:, S2_BET:S2_BET + 256]

            # ---------------- main loop (software pipelined) ----------------
            def emit_front(ch):
                """relu/mask + projections (v, s_k, g, g*v) for chunk ch."""
                wt4 = wt_tiles[ch]
                W1c = elem.tile([128, CJT, ISLICE], bf, tag="W1",
                                name=f"W1_{ch}")
                nc.scalar.activation(W1c, wt4, AF.Relu)
                mskc = elem.tile([128, CJT, ISLICE], bf, tag="msk",
                                 name=f"msk_{ch}")
                nc.vector.tensor_scalar(mskc, wt4, 0.0, None, op0=OP.is_gt)

                rhs4 = rhsp.tile([128, CJT, 260], bf, tag="rhs",
                                 name=f"rhs_{ch}")
                for jm in range(CJT):
                    jt = ch * CJT + jm
                    ps_vk = prep.tile([128, 260], f32, tag="vk",
                                      name=f"vk_{jt % 4}")
                    for a in range(2):
                        nc.tensor.matmul(ps_vk, ht_sb[:, a, ts(jt, 128)],
                                         su1[:, a, :],
                                         start=(a == 0), stop=(a == 1))
                    # g = exp(s_k) written straight into rhs cols 256:260
                    nc.scalar.activation(rhs4[:, jm, 256:260],
                                         ps_vk[:, 256:260], AF.Exp)
                    gsl = rhs4[:, jm, 256:260]
                    gb = bass.AP(tensor=gsl.tensor, offset=gsl.offset,
                                 ap=[gsl.ap[0], gsl.ap[1], [0, DH]])
                    nc.vector.tensor_tensor(
                        out=rhs4[:, jm, 0:256].rearrange(
                            "p (h d) -> p h d", h=H),
                        in0=ps_vk[:, 0:256].rearrange("p (h d) -> p h d", h=H),
                        in1=gb, op=OP.mult)
                return W1c, mskc, rhs4

            def emit_accum(ch, W1c, mskc, rhs4):
                st = (ch == 0)
                sp = (ch == NCH - 1)
                for jm in range(CJT):
                    for s in range(NSUB):
                        nc.tensor.matmul(psA[s][:, 0:260],
                                         W1c[:, jm, ts(s, 128)],
                                         rhs4[:, jm, :], start=st, stop=sp,
                                         skip_group_check=True)
                for jm in range(CJT):
                    for s in range(NSUB):
                        nc.tensor.matmul(psA[s][:, 260:264],
                                         mskc[:, jm, ts(s, 128)],
                                         rhs4[:, jm, 256:260], start=st,
                                         stop=sp, skip_group_check=True)

            prev = emit_front(0)
            for ch in range(1, NCH):
                cur = emit_front(ch)
                emit_accum(ch - 1, *prev)
                prev = cur
            emit_accum(NCH - 1, *prev)

            # ---------------- epilogue ----------------
            rdens, agrs = [], []
            for s in range(NSUB):
                den = small.tile([128, H], f32, tag="den", name=f"den{s}")
                nc.vector.tensor_tensor(den, psA[s][:, 256:260], c1b_sb,
                                        op=OP.mult)
                nc.vector.tensor_add(den, den, psA[s][:, 260:264])
                rden = small.tile([128, H], f32, tag="rden", name=f"rden{s}")
                nc.vector.reciprocal(rden, den)
                agr = small.tile([128, H], f32, tag="agr", name=f"agr{s}")
                nc.vector.tensor_mul(agr, psA[s][:, 256:260], rden)
                rdens.append(rden)
                agrs.append(agr)

            msgs = []
            for s in range(NSUB):
                # per-head scaling on the scalar engine keeps DVE free:
                # msg_h = Num_h * rden_h ; msgb_h = bv_h * (Ag_h * rden_h)
                msg = outp.tile([128, D], bf, tag="msg", name=f"msg{s}")
                msgb = outp.tile([128, D], bf, tag="msgb", name=f"msgb{s}")
                for hh in range(H):
                    hsl = slice(hh * DH, (hh + 1) * DH)
                    nc.scalar.mul(msg[:, hsl], psA[s][:, hsl],
                                  rdens[s][:, hh:hh + 1])
                    nc.scalar.mul(msgb[:, hsl], bv_sb[:, hsl],
                                  agrs[s][:, hh:hh + 1])
                msg2 = outp.tile([128, D], bf, tag="msg2", name=f"msg2{s}")
                nc.vector.tensor_add(msg2, msg, msgb)
                msgs.append(msg2)

            msgTs = []
            for s in range(NSUB):
                ps_t = prep.tile([128, 2, 128], bf, tag="vk", name=f"pst{s}")
                for b in range(2):
                    nc.tensor.transpose(ps_t[:, b, :], msgs[s][:, ts(b, 128)],
                                        ident)
                msgT = outp.tile([128, 2, 128], bf, tag="msgT",
                                 name=f"msgT{s}")
                nc.vector.tensor_copy(msgT, ps_t)
                msgTs.append(msgT)

            for s in range(NSUB):
                ps_o = prep.tile([128, D], f32, tag="vk", name=f"pso{s}")
                nc.tensor.matmul(ps_o, msgTs[s][:, 0, :], WoT_sb[:, 0, :],
                                 start=True, stop=False)
                nc.tensor.matmul(ps_o, msgTs[s][:, 1, :], WoT_sb[:, 1, :],
                                 start=False, stop=False)
                nc.tensor.matmul(ps_o, ones_sb, bo_row, start=False,
                                 stop=True)

                x = outp.tile([128, D], f32, tag="x", name=f"x{s}")
                nc.vector.tensor_add(x, ps_o, hseg_all[:, s, :])

                stats = small.tile([128, 6], f32, tag="stats", name=f"st{s}")
                nc.vector.bn_stats(out=stats, in_=x)
                mv = small.tile([128, 2], f32, tag="mv", name=f"mv{s}")
                nc.vector.bn_aggr(out=mv, in_=stats)
                sd = small.tile([128, 1], f32, tag="sd", name=f"sd{s}")
                nc.scalar.activation(sd, mv[:, 1:2], AF.Sqrt, bias=eps_sb)
                rstd = small.tile([128, 1], f32, tag="rstd", name=f"rst{s}")
                nc.vector.reciprocal(rstd, sd)

                # G = gamma * rstd  (per-partition scalar mult, scalar engine)
                G = outp.tile([128, D], f32, tag="G", name=f"G{s}")
                nc.scalar.mul(G, gam_sb, rstd)
                # ot = (x - mu) * G
                ot = outp.tile([128, D], f32, tag="ot", name=f"ot{s}")
                nc.vector.scalar_tensor_tensor(
                    out=ot, in0=x, scalar=mv[:, 0:1], in1=G,
                    op0=OP.subtract, op1=OP.mult)
                otb = outp.tile([128, D], f32, tag="otb", name=f"otb{s}")
                nc.vector.tensor_add(otb, ot, bet_sb)
                nc.sync.dma_start(out_d[ts(s, 128), :], otb)

    nc.compile()
    return nc


def _make_in_maps(h, w, Wk, Wv, bv, We_w, u, Wo, bo, gamma, beta, **_unused):
    import ml_dtypes
    f = np.float32
    b16 = ml_dtypes.bfloat16
    h32 = np.ascontiguousarray(h, dtype=f)
    wT = np.asarray(w, dtype=f).T.astype(b16)
    Wk = np.asarray(Wk, dtype=f)
    u = np.asarray(u, dtype=f)
    We_w = np.asarray(We_w, dtype=f)

    # host-folded weight constants
    u_k, u_e = u[:, DH:2 * DH], u[:, 2 * DH:2 * DH + DE]
    a_k = np.stack([u_k[hh] @ Wk[hh * DH:(hh + 1) * DH, :]
                    for hh in range(H)], axis=1)          # [256, 4]
    c1 = np.array([We_w[hh * DE:(hh + 1) * DE, 0] @ u_e[hh]
                   for hh in range(H)], dtype=f)          # [4]

    # su1 [128, 2, 260]: per d-half a: WvT block | a_k block
    su1 = np.zeros((128, 2, 260), f)
    WvT = np.asarray(Wv, dtype=f).T                        # [din, dout]
    su1[:, :, 0:256] = WvT.reshape(2, 128, D).transpose(1, 0, 2)
    su1[:, :, 256:260] = a_k.reshape(2, 128, H).transpose(1, 0, 2)

    # su2 [128, S2_COLS]
    su2 = np.zeros((128, S2_COLS), f)
    WoT = np.asarray(Wo, dtype=f).T
    su2[:, S2_WOT:S2_WOT + 512] = WoT.reshape(2, 128, D).transpose(
        1, 0, 2).reshape(128, 512)
    su2[:, S2_ID:S2_ID + 128] = np.eye(128, dtype=f)
    su2[0, S2_BO:S2_BO + 256] = np.asarray(bo, dtype=f)
    su2[:, S2_C1:S2_C1 + 4] = c1[None, :]
    su2[:, S2_BV:S2_BV + 256] = np.asarray(bv, dtype=f)[None, :]
    su2[:, S2_GAM:S2_GAM + 256] = np.asarray(gamma, dtype=f)[None, :]
    su2[:, S2_BET:S2_BET + 256] = np.asarray(beta, dtype=f)[None, :]

    common = {
        "ht": np.ascontiguousarray(h32.T.astype(b16)),
        "su1": su1.reshape(128, 520).astype(b16),
        "su2": su2.astype(b16),
    }
    in_maps = []
    for c in range(NCORES):
        sl = slice(c * ISLICE, (c + 1) * ISLICE)
        m = dict(common)
        m["wt"] = np.ascontiguousarray(wT[:, sl])
        m["hs"] = np.ascontiguousarray(h32[sl, :])
        in_maps.append(m)
    return in_maps


def kernel(**inputs):
    from concourse.bass_utils import run_bass_kernel_spmd

    if "nc" not in _cache:
        _cache["nc"] = _build_bass()
    nc = _cache["nc"]

    in_maps = _make_in_maps(**inputs)
    res = run_bass_kernel_spmd(nc, in_maps, core_ids=list(range(NCORES)))
    out = np.concatenate([r["out"] for r in res.results], axis=0)
    return np.ascontiguousarray(out, dtype=np.float32)
